# revision 8
# baseline (speedup 1.0000x reference)
"""Expert-parallel MoE FFN kernel for Trainium2 (8 NeuronCores, one expert per core).

Host side: routes tokens to experts (dedup per expert, summing duplicate top-k
weights), pads each expert's token list to a common T_PAD (multiple of 32,
sized to the max per-expert count), and pre-tiles the weight matrices into
DMA-friendly contiguous blocks.

Device side (per core, expert e):
  h^T = silu(G_e^T X^T) * (U_e^T X^T)     [I, T]   (stage A)
  y   = (h^T)^T @ D_e  * cw               [T, H]   (stage B; h^T tiles are the
                                           stationary operand so the cw combine
                                           becomes a per-partition scale)
All matmuls are bf16 with fp32 PSUM accumulation.

Perf structure:
 - Short HAM warm-up (3 dummy matmuls on a Vector-memset scratch tile, first
   issue ~7.5us) so the PE's HAM activity window opens as early as possible;
   the first real matmul lands right as its data arrives, mostly warm.
 - The head-critical inputs are split fine-grained (x0 per 512-token chunk,
   gate0/up0 per 4-k-tile half) and spread across engine DMA queues, so the
   first matmul gates on ~250KB instead of ~750KB of DMA.
 - cw is DMAed after the gate/up prefetch stream (it is only read by the
   stage-B drain), keeping the head rings free for the PE-critical tiles.
 - One shared PSUM pool (tags q0..q3, bufs=2) spans both stages, so stage B's
   accumulators take over stage A's bank ring without a pool barrier.
 - Stage B drains per 512-column PSUM chunk (cw scale on alternating Scalar/
   Vector engines, immediate per-chunk y DMA on rotating queues); the final
   chunk drains as two 256-column pieces on parallel engine pairs so the
   post-matmul scale->descgen->ring tail is halved. The NEFF's fixed
   semaphore-teardown epilogue starts right after the last y DMA, so tail ns
   are end-to-end ns.
 - A post-schedule pass drops LDWEIGHTS reloads whose stationary tile is
   already in the PE array, and another hoists prefetch-satisfied semaphore
   waits off PE instructions so LDWEIGHTS stays eligible for pull-ahead.
"""
import sys

if "/opt/trn_rl_repo" not in sys.path:
    sys.path.insert(0, "/opt/trn_rl_repo")

import numpy as np

N_TOKENS, TOP_K, N_EXPERTS, HIDDEN, INTER = 4096, 2, 8, 1024, 2048
P = 128
NI = INTER // P          # 16 I-tiles
KH = HIDDEN // P         # 8 H(contraction)-tiles
NHC = HIDDEN // 512      # 2 output-column chunks
N_WARM = 3               # HAM warm-up matmuls (N=512 each, ~0.43us cold)

_CACHE = {}


def _build(t_pad):
    import concourse.bacc as bacc
    import concourse.mybir as mybir
    import concourse.tile as tile

    f32 = mybir.dt.float32
    bf16 = mybir.dt.bfloat16

    nt = (t_pad + P - 1) // P          # t-blocks of 128 (last may be partial)
    # stage-A free-dim chunks of <=512 covering t_pad
    chunks = [(c, min(c + 512, t_pad)) for c in range(0, t_pad, 512)]
    ntc = len(chunks)
    assert ntc <= 2, "PSUM tag layout assumes at most 2 token chunks"

    nc = bacc.Bacc()
    xt = nc.declare_dram_parameter("xt", [KH, P, t_pad], bf16, isOutput=False)
    gw = nc.declare_dram_parameter("gw", [NI, P, HIDDEN], bf16, isOutput=False)
    uw = nc.declare_dram_parameter("uw", [NI, P, HIDDEN], bf16, isOutput=False)
    dw = nc.declare_dram_parameter("dw", [NI, P, HIDDEN], bf16, isOutput=False)
    cw = nc.declare_dram_parameter("cw", [P, nt], f32, isOutput=False)
    y = nc.declare_dram_parameter("y", [t_pad, HIDDEN], bf16, isOutput=True)

    with tile.TileContext(nc) as tc:
        with (
            tc.tile_pool(name="xp", bufs=1) as xp,
            tc.tile_pool(name="hp", bufs=1) as hp,
            tc.tile_pool(name="wp", bufs=4) as wp,
            tc.tile_pool(name="dp", bufs=1) as dp,
            tc.tile_pool(name="cp", bufs=1) as cp,
            tc.tile_pool(name="gp", bufs=4) as gp,
            tc.tile_pool(name="ep", bufs=3) as ep,
            tc.tile_pool(name="sp", bufs=1) as sp,
            tc.tile_pool(name="ps", bufs=2, space="PSUM") as ps,
        ):
            # ---- HAM warm-up: a few dummy matmuls on a Vector-memset
            # scratch tile. Vector's preamble is the shortest of the five
            # engines, so the first warm-up matmul lands ~7.5us -- opening
            # the PE's HAM activity window ~3.4us before the first real
            # matmul needs full clock.
            scratch = sp.tile([P, 512], bf16, name="scratch")
            nc.vector.memset(scratch[:], 0)
            wps = ps.tile([P, 512], f32, tag="q0", name="warm")
            for _ in range(N_WARM):
                nc.tensor.matmul(out=wps[:], lhsT=scratch[:, 0:P],
                                 rhs=scratch[:], start=True, stop=True)

            # ---- Input tiles. The k=0 x tile and the i=0 gate/up tiles are
            # split so the first matmul gates on small transfers; everything
            # else streams whole tiles.
            x0s = [xp.tile([P, c1 - c0], bf16, tag=f"x0_{c}", name=f"xt0_{c}")
                   for c, (c0, c1) in enumerate(chunks)]
            xts = [None] + [xp.tile([P, t_pad], bf16, tag=f"x{k}",
                                    name=f"xt{k}") for k in range(1, KH)]
            g0s = [wp.tile([P, 512], bf16, tag=f"g0{h}", name=f"gt0{h}")
                   for h in range(2)]
            u0s = [wp.tile([P, 512], bf16, tag=f"u0{h}", name=f"ut0{h}")
                   for h in range(2)]

            def xap(k, c):
                c0, c1 = chunks[c]
                return x0s[c][:] if k == 0 else xts[k][:, c0:c1]

            def wslice(ts, half_tiles, k):
                # k-slice [P, 128] of a [P, HIDDEN] weight tile (or its
                # split halves for i=0)
                if half_tiles is not None:
                    return half_tiles[k // 4][:, (k % 4) * P:(k % 4 + 1) * P]
                return ts[:, k * P:(k + 1) * P]

            # ---- DMA issue order: head-critical first, on parallel queues.
            nc.sync.dma_start(out=x0s[0][:], in_=xt[0][:, 0:512])
            nc.scalar.dma_start(out=g0s[0][:], in_=gw[0][:, 0:512])
            nc.gpsimd.dma_start(out=u0s[0][:], in_=uw[0][:, 0:512])
            if ntc > 1:
                nc.sync.dma_start(out=x0s[1][:], in_=xt[0][:, 512:t_pad])
            nc.scalar.dma_start(out=g0s[1][:], in_=gw[0][:, 512:HIDDEN])
            nc.gpsimd.dma_start(out=u0s[1][:], in_=uw[0][:, 512:HIDDEN])
            for k in range(1, KH):
                nc.sync.dma_start(out=xts[k][:], in_=xt[k])
            wpre = [(g0s, u0s)]
            for i in range(1, 4):
                gt = wp.tile([P, HIDDEN], bf16, tag="g", name=f"gt{i}")
                ut = wp.tile([P, HIDDEN], bf16, tag="u", name=f"ut{i}")
                nc.sync.dma_start(out=gt[:], in_=gw[i])
                nc.sync.dma_start(out=ut[:], in_=uw[i])
                wpre.append((gt, ut))
            dts = [dp.tile([P, HIDDEN], bf16, tag=f"d{i}", name=f"dt{i}")
                   for i in range(NI)]
            cwt = cp.tile([P, nt], f32, name="cwt")
            nc.scalar.dma_start(out=cwt[:], in_=cw[:])

            hts = [hp.tile([P, t_pad], bf16, tag=f"h{i}", name=f"ht{i}")
                   for i in range(NI)]

            # ---- Stage A: h^T[i] = silu(G^T X^T) * (U^T X^T), tiled over I ----
            for i in range(NI):
                ghalf = uhalf = None
                if i == 0:
                    (ghalf, uhalf), gt, ut = wpre[0], None, None
                elif i < 4:
                    gt, ut = wpre[i]
                else:
                    gt = wp.tile([P, HIDDEN], bf16, tag="g", name=f"gt{i}")
                    ut = wp.tile([P, HIDDEN], bf16, tag="u", name=f"ut{i}")
                    nc.sync.dma_start(out=gt[:], in_=gw[i])
                    nc.sync.dma_start(out=ut[:], in_=uw[i])
                nc.sync.dma_start(out=dts[i][:], in_=dw[i])
                pgs = [ps.tile([P, 512], f32, tag=f"q{c}", name=f"pg{i}_{c}")
                       for c in range(ntc)]
                pus = [ps.tile([P, 512], f32, tag=f"q{ntc + c}", name=f"pu{i}_{c}")
                       for c in range(ntc)]
                for k in range(KH):
                    lg = wslice(gt, ghalf, k)
                    lu = wslice(ut, uhalf, k)
                    for c, (c0, c1) in enumerate(chunks):
                        nc.tensor.matmul(out=pgs[c][:, :c1 - c0], lhsT=lg,
                                         rhs=xap(k, c),
                                         start=(k == 0), stop=(k == KH - 1))
                    for c, (c0, c1) in enumerate(chunks):
                        nc.tensor.matmul(out=pus[c][:, :c1 - c0], lhsT=lu,
                                         rhs=xap(k, c),
                                         start=(k == 0), stop=(k == KH - 1))
                for c, (c0, c1) in enumerate(chunks):
                    w = c1 - c0
                    sg = gp.tile([P, 512], f32, tag="sg", name="sg")
                    nc.scalar.activation(out=sg[:, :w], in_=pgs[c][:, :w],
                                         func=mybir.ActivationFunctionType.Silu)
                    nc.vector.tensor_mul(out=hts[i][:, c0:c1],
                                         in0=sg[:, :w], in1=pus[c][:, :w])

            # ---- Stage B: y[tb,:] = sum_i (h^T tile)^T @ D[i], scaled by cw.
            # h^T tiles are stationary; D rows stream. Output is y [T, H], so
            # cw is a per-partition scalar (Scalar-engine scale / DVE
            # tensor_scalar) applied per 512-column chunk as soon as that
            # chunk's accumulation stops, with an immediate per-chunk DMA.
            # The very last chunk drains as two 256-column pieces on
            # parallel engine pairs to halve the end-of-kernel tail.
            dma_engs = [nc.sync, nc.scalar, nc.gpsimd]
            ngroups = (nt + 1) // 2
            for g in range(ngroups):
                tbs = [tb for tb in (2 * g, 2 * g + 1) if tb < nt]
                rows = [min(P, t_pad - tb * P) for tb in tbs]
                pys = [[ps.tile([P, 512], f32, tag=f"q{ti * NHC + hc}",
                                name=f"py{g}_{ti}_{hc}")
                        for hc in range(NHC)] for ti in range(len(tbs))]
                for i in range(NI):
                    for ti, tb in enumerate(tbs):
                        r = rows[ti]
                        lh = hts[i][:, tb * P:tb * P + r]
                        for hc in range(NHC):
                            nc.tensor.matmul(
                                out=pys[ti][hc][:r, :], lhsT=lh,
                                rhs=dts[i][:, hc * 512:(hc + 1) * 512],
                                start=(i == 0), stop=(i == NI - 1))
                for ti, tb in enumerate(tbs):
                    r = rows[ti]
                    for hc in range(NHC):
                        last = (g == ngroups - 1 and ti == len(tbs) - 1
                                and hc == NHC - 1)
                        pieces = ((0, 256), (256, 512)) if last else ((0, 512),)
                        for pi, (p0, p1) in enumerate(pieces):
                            tag = f"yz{pi}" if last else f"y{ti}{hc}"
                            ybt = ep.tile([P, p1 - p0], bf16, tag=tag,
                                          name=f"ybt{tag}")
                            if (hc + pi) % 2 == 0:
                                nc.scalar.activation(
                                    out=ybt[:r, :], in_=pys[ti][hc][:r, p0:p1],
                                    func=mybir.ActivationFunctionType.Copy,
                                    scale=cwt[:r, tb:tb + 1])
                            else:
                                nc.vector.tensor_scalar_mul(
                                    ybt[:r, :], pys[ti][hc][:r, p0:p1],
                                    cwt[:r, tb:tb + 1])
                            eng = dma_engs[(ti * NHC + hc + pi) % 3]
                            eng.dma_start(
                                out=y[tb * P:tb * P + r,
                                      hc * 512 + p0:hc * 512 + p1],
                                in_=ybt[:r, :])

    _dedup_ldweights(nc)
    _hoist_pe_waits(nc)
    nc.finalize()
    return nc


def _hoist_pe_waits(nc, dist=8, skip=52):
    """Move semaphore waits off PE matmul/ldweights instructions into a
    standalone EVENT_SEMAPHORE `dist` engine-instructions earlier. A bare
    LDWEIGHTS can be pulled ahead of in-flight matmuls by the PE's reorder
    window; a wait-carrying one cannot (measured: 432ns vs 213ns pacing at
    every weight-ring boundary). All hoisted waits are prefetch-satisfied
    long before the insertion point. The first `skip` engine instructions
    (warm-up + first i-iteration, input DMA still in flight) keep their
    waits in place."""
    import concourse.mybir as mybir
    from collections import defaultdict

    for blk in nc.m.functions[0].blocks:
        pe_pos = [bi for bi, i in enumerate(blk.instructions)
                  if getattr(i, "engine", None) == mybir.EngineType.PE
                  and isinstance(i, (mybir.InstMatmult, mybir.InstLdweights))]
        if len(pe_pos) < skip:
            continue
        inserts = []  # (block_index, evsem)
        for j, bi in enumerate(pe_pos):
            if j < skip:
                continue
            inst = blk.instructions[bi]
            si = inst.sync_info
            if not (si and si.on_wait):
                continue
            target = pe_pos[max(skip, j - dist)]
            if target >= bi:
                continue
            for w in si.on_wait:
                # DMA-completion waits (input prefetches, always long
                # satisfied) go `dist` engine-instructions early; engine-sem
                # waits (PSUM WAR etc.) split to an EVSEM immediately before
                # the instruction -- same queue position, but the LDW/MM
                # itself becomes bare and eligible for pull-ahead.
                t = target if "DMA" in (w.ant_name or "") else bi
                ev = mybir.InstEventSemaphore(
                    name=nc.get_next_instruction_name(), ins=[], outs=[])
                ev.engine = mybir.EngineType.PE
                ev.sync_info = mybir.SyncInfo(on_wait=[w], on_update=[])
                nc.register_instruction(ev)
                inserts.append((t, ev))
            si.on_wait = []
        if not inserts:
            continue
        by_idx = defaultdict(list)
        for t, ev in inserts:
            by_idx[t].append(ev)
        out = []
        for bi, inst in enumerate(blk.instructions):
            if bi in by_idx:
                out.extend(by_idx[bi])
            out.append(inst)
        blk.instructions[:] = out


def _dedup_ldweights(nc):
    """Drop an InstLdweights whose weights AP matches the immediately
    preceding load on the PE queue (matmuls between don't clobber the
    array). Saves the ~46ns/matmul the redundant reload steals from the
    PE issue pipeline. Only sync-free duplicates are removed."""
    import concourse.mybir as mybir

    def key(i):
        a = i.ins[0]
        return (a.memref, a.offset, tuple(map(tuple, a.ap)), str(a.dtype),
                str(i.perf_mode), str(i.is_transpose),
                str(getattr(i, "tile_position", None)))

    for blk in nc.m.functions[0].blocks:
        last = None
        keep = []
        for i in blk.instructions:
            if getattr(i, "engine", None) == mybir.EngineType.PE:
                if isinstance(i, mybir.InstLdweights):
                    k = key(i)
                    si = i.sync_info
                    clean = not (si and (si.on_wait or si.on_update))
                    if k == last and clean:
                        continue
                    last = k
                elif not isinstance(i, mybir.InstMatmult):
                    last = None
            keep.append(i)
        blk.instructions[:] = keep


def _route(expert_indices, expert_weights):
    idx = np.asarray(expert_indices).astype(np.int64)
    wts = np.asarray(expert_weights).astype(np.float32)
    n = idx.shape[0]
    cw_full = np.zeros((N_EXPERTS, n), np.float32)
    for k in range(idx.shape[1]):
        np.add.at(cw_full, (idx[:, k], np.arange(n)), wts[:, k])
    ids = [np.nonzero(cw_full[e])[0] for e in range(N_EXPERTS)]
    maxc = max(len(i) for i in ids)
    t_pad = max(512, ((maxc + 31) // 32) * 32)
    return cw_full, ids, t_pad


def _run(nc, in_maps, trace=False, trace_cores=None):
    from concourse.bass_utils import run_bass_kernel_spmd

    return run_bass_kernel_spmd(
        nc, in_maps, list(range(N_EXPERTS)), trace=trace,
        trace_cores=trace_cores,
    )


def prepare(tokens, expert_indices, expert_weights, gate_weight, up_weight,
            down_weight):
    """Host-side routing + layout. Returns (nc, in_maps, ids, t_pad)."""
    tokens = np.ascontiguousarray(np.asarray(tokens, dtype=np.float32))
    gate_weight = np.asarray(gate_weight, dtype=np.float32)
    up_weight = np.asarray(up_weight, dtype=np.float32)
    down_weight = np.asarray(down_weight, dtype=np.float32)

    cw_full, ids, t_pad = _route(expert_indices, expert_weights)
    nt = (t_pad + P - 1) // P

    key = t_pad
    if key not in _CACHE:
        _CACHE[key] = _build(t_pad)
    nc = _CACHE[key]

    mmdt = np.dtype("bfloat16")
    in_maps = []
    for e in range(N_EXPERTS):
        ce = len(ids[e])
        xe = np.zeros((HIDDEN, t_pad), np.float32)
        xe[:, :ce] = tokens[ids[e]].T
        cwe = np.zeros((nt * P,), np.float32)
        cwe[:ce] = cw_full[e, ids[e]]
        in_maps.append({
            "xt": np.ascontiguousarray(xe.reshape(KH, P, t_pad)).astype(mmdt),
            "gw": np.ascontiguousarray(
                gate_weight[e].reshape(KH, P, NI, P).transpose(2, 1, 0, 3)
            ).reshape(NI, P, HIDDEN).astype(mmdt),
            "uw": np.ascontiguousarray(
                up_weight[e].reshape(KH, P, NI, P).transpose(2, 1, 0, 3)
            ).reshape(NI, P, HIDDEN).astype(mmdt),
            "dw": np.ascontiguousarray(down_weight[e].reshape(NI, P, HIDDEN)).astype(mmdt),
            "cw": np.ascontiguousarray(cwe.reshape(nt, P).T),
        })
    return nc, in_maps, ids, t_pad


def combine(results, ids):
    out = np.zeros((N_TOKENS, HIDDEN), np.float32)
    for e in range(N_EXPERTS):
        ce = len(ids[e])
        out[ids[e]] += results[e]["y"][:ce].astype(np.float32)
    return out


def kernel(tokens, expert_indices, expert_weights, gate_weight, up_weight,
           down_weight):
    nc, in_maps, ids, _ = prepare(tokens, expert_indices, expert_weights,
                                  gate_weight, up_weight, down_weight)
    res = _run(nc, in_maps, trace=False)
    return combine(res.results, ids)
